# revision 43
# baseline (speedup 1.0000x reference)
r"""DetCon (NT-Xent style) contrastive loss on 8 Trainium2 NeuronCores.

Reference computes, for v0/v1 = L2-normalized (over E) views scaled by
1/sqrt(T):   logits = [[S01, S00\diag], [S10, S11\diag]]  (2BN x 2BN-1)
             loss = mean_i( logsumexp(row_i) - label_logit_i )
with label_logit_i = S01[i,i] (== S10[i,i]).

v3: TRANSPOSED tiles. Each core owns 1024 rows (512 per view); logits are
computed as S^T tiles [128 key-cols (partitions), 1024 rows (free)], so:
  - keys stay RAW fp8 (one gpsimd copy, no per-column normalize multiply)
  - the key-side normalize scale s_j is per-PARTITION: folded into the
    exp for free (ACT `scale` arg / DVE Schraudolph per-partition scalar)
  - the row-side scale s_r is folded into the fp8 moving operand (rows8)
  - row-sums of exp become PARTITION sums -> tiny PE ones-matmuls
    (fp8 DoubleRow for ACT tiles, bf16 for DVE Schraudolph tiles),
    accumulated in two [1, 512] PSUM rows; no ACT accum reads, no DVE
    second pass
  - scales are computed COMPACTLY: sum-of-squares rows -> PSUM [1,512]
    chunks -> SBUF -> PE-transpose -> [128, nn] -> ACT Ln+Exp
    ((0.1*ss)^-0.5) in the same activation table as the main Exp --
    zero activation-table reloads, no DVE reciprocal
  - same-view diagonal removed EXACTLY by zero-masking 128x128 diag
    blocks of the exp tiles (gpsimd); label logits extracted from PSUM
    via DVE stst+identity then scaled by sclT
Host sums the 8 per-core partial sums (+5 fold-back) and divides by 2BN.
"""

import math
from contextlib import ExitStack

import numpy as np

import concourse.bacc as bacc
import concourse.bass as bass
import concourse.tile as tile
from concourse import mybir
from concourse.bass_utils import run_bass_kernel_spmd
from concourse.hw_specs import get_activation_tables

B, E, N = 64, 256, 64
BN = B * N            # 4096 columns per view
NCORES = 8
CHUNK = BN // NCORES  # 512 rows (of each view) per core
ROWS = 2 * CHUNK      # 1024 moving rows per core
P = 128
KH = E // P           # 2 contraction halves
LG = 8                # load groups (512 columns each)
GL = BN // LG         # 512
GB = B // LG          # 8 b-slices per load group
EXPB = -5.0           # exp bias: tiles hold e^(l-5) so fp8 never overflows

# bf16 Schraudolph exp: bits(e^x) ~= int16(x * 184.66 + (16256 + C))
SCH_A = 184.6649652337873
SCH_C = -5.0
SCH_B = 16256.0 + SCH_C + EXPB * SCH_A

F32 = mybir.dt.float32
BF16 = mybir.dt.bfloat16
FP8 = mybir.dt.float8e4
I16 = mybir.dt.int16
AF = mybir.ActivationFunctionType

# ACT share of the 32 jb pairs
ACT_PAIRS = 18

# per-load-group scale regions: lg_k's key scales are ready ~1.5us after
# lg_k's own load lands (no cross-group gating)
SG_LGS = tuple((lg,) for lg in range(LG))


def _cc(v, lg, q):
    """Compact scale column for view v, load-group lg, 128-chunk q."""
    return lg * 8 + v * 4 + q


def _build_schedule():
    """Work units for the main phase: ACT units are jb PAIRS (so the two
    fp8 exp tiles share one DoubleRow ones-matmul); DVE units are single
    jbs (bf16 ones-matmuls) interleaved between ACT pairs so DVE never
    starves on the 2-deep pt ring. Both queues sweep jc (and hence load
    groups) in order; merged by finish time."""
    pairs = [(i, kv) for i in range(16) for kv in range(2)]
    asg = {}
    qa = qv = 0
    va, vv = ACT_PAIRS, 32 - ACT_PAIRS
    for p in pairs:
        if vv * qa <= va * qv:
            asg[p] = 'A'
            qa += 1
        else:
            asg[p] = 'V'
            qv += 1
    ua = [(kv, (2 * i, 2 * i + 1)) for (i, kv) in pairs if asg[(i, kv)] == 'A']
    uv = [(kv, (jc,)) for (i, kv) in pairs if asg[(i, kv)] == 'V'
          for jc in (2 * i, 2 * i + 1)]
    out = []
    ta = tv = 0.0
    ia = iv = 0
    TA, TV = 2.1, 1.35
    while ia < len(ua) or iv < len(uv):
        if iv >= len(uv) or (ia < len(ua) and ta + TA <= tv + TV):
            out.append(('A',) + ua[ia])
            ia += 1
            ta += TA
        else:
            out.append(('V',) + uv[iv])
            iv += 1
            tv += TV
    return out


SCHEDULE = _build_schedule()


def _emit_loads(nc, pl, vin, r):
    """view1 lg0/lg1 ride the ACT queue (needed first, before any ACT
    compute); everything else queues on SP in lg order so the SP wall
    (~44us) matches the exp-work wall."""
    raw = [pl["raw"].tile([P, KH, BN], F32, tag=f"raw{v}", name=f"raw{v}_{r}")
           for v in range(2)]
    for lg in range(LG):
        for v in range(2):
            for h in range(KH):
                src = vin[v][lg * GB:(lg + 1) * GB, h * P:(h + 1) * P, :] \
                    .rearrange("b e n -> e b n")
                dst = raw[v][:, h, lg * GL:(lg + 1) * GL] \
                    .rearrange("e (b n) -> e b n", b=GB)
                eng = nc.scalar if v == 1 else nc.sync
                eng.dma_start(out=dst, in_=src)
    return raw


_LG_SG = {lg: (lg, 0) for lg in range(LG)}


def _emit_norm_lg(nc, pl, r, v, lg, raw, keys8, ssc_sb, bc_out=None):
    """Per (view, load-group): raw->fp8 keys, squares, sumsq, compact copy.
    For lg0 also emits a broadcast sumsq (M=128) used by the rows8 path."""
    ones8_1 = pl["consts"]["ones8_1"]
    gs = slice(lg * GL, (lg + 1) * GL)
    sq = pl["sq"].tile([P, KH, GL], FP8, tag="sq", name=f"sq{v}{lg}_{r}")
    for h in range(KH):
        nc.gpsimd.tensor_copy(keys8[v][:, h, gs], raw[v][:, h, gs])
        nc.gpsimd.tensor_mul(sq[:, h, :], raw[v][:, h, gs], raw[v][:, h, gs])
    if bc_out is not None:
        nc.tensor.matmul(bc_out[v][:], pl["consts"]["ones8F"][:], sq[:, :, :],
                         perf_mode=mybir.MatmulPerfMode.DoubleRow)
    ssc = pl["pt"].tile([32, GL], F32, tag="pt", name=f"ssc{v}{lg}_{r}")
    nc.tensor.matmul(ssc[:], ones8_1[:], sq[:, :, :],
                     perf_mode=mybir.MatmulPerfMode.DoubleRow)
    t = pl["sml"].tile([1, GL], F32, tag=f"sscb{v}{lg}", name=f"sscb{v}{lg}_{r}")
    nc.vector.tensor_copy(t[:], ssc[0:1, :])
    ssc_sb[(v, lg)] = t


def _emit_scale_sg(nc, pl, r, sg, ssc_sb, sclT, sAT):
    """Transpose compact ss chunks and compute sclT = (0.1*ss)^-0.5."""
    ident = pl["consts"]["ident"]
    lgs = SG_LGS[sg]
    nn = 2 * 4 * len(lgs)
    pssT = pl["pt"].tile([P, nn], F32, tag="pt", name=f"pssT{sg}_{r}")
    cc0 = _cc(0, lgs[0], 0)
    for v in range(2):
        for lg in lgs:
            for q in range(4):
                col = _cc(v, lg, q) - cc0
                nc.tensor.transpose(
                    pssT[:, col:col + 1],
                    ssc_sb[(v, lg)][0:1, q * P:(q + 1) * P],
                    ident[0:1, 0:1])
    lnt = pl["sml"].tile([P, nn], F32, tag=f"lnt{sg}", name=f"lnt{sg}_{r}")
    nc.scalar.activation(lnt[:], pssT[:], AF.Ln, scale=0.1)
    nc.scalar.activation(sclT[:, cc0:cc0 + nn], lnt[:], AF.Exp, scale=-0.5)
    nc.vector.tensor_scalar(sAT[:, cc0:cc0 + nn], sclT[:, cc0:cc0 + nn],
                            SCH_A, 0.0, op0=mybir.AluOpType.mult,
                            op1=mybir.AluOpType.add)


def _emit_rows8(nc, pl, r, raw, ssb_bc, rows8):
    """rows8[:, h, v*512:(v+1)*512] = raw_rows * s_row (fp8). The row
    scale comes straight from the broadcast sumsq via ACT Ln+Exp --
    no transpose/copy/broadcast detour on the critical path."""
    for v in range(2):
        lnb = pl["sml"].tile([P, GL], F32, tag=f"lnb{v}", name=f"lnb{v}_{r}")
        nc.scalar.activation(lnb[:], ssb_bc[v][:], AF.Ln, scale=0.1)
        sclb = pl["sml"].tile([P, GL], BF16, tag=f"sclb{v}",
                              name=f"sclb{v}_{r}")
        nc.scalar.activation(sclb[:], lnb[:], AF.Exp, scale=-0.5)
        for h in range(KH):
            nc.vector.tensor_mul(rows8[:, h, v * CHUNK:(v + 1) * CHUNK],
                                 raw[v][:, h, 0:CHUNK], sclb[:])


def _emit_unit(nc, pl, r, u, eng, kv, jcs, keys8, rows8, sclT, sAT,
               rsA, rsB, diag01, first, last):
    """One work unit: logits, exp, (mask/label), rowsum matmuls."""
    ident = pl["consts"]["ident"]
    ones8_1 = pl["consts"]["ones8_1"]
    ones1b = pl["consts"]["ones1b"]
    bias5 = pl["consts"]["bias5"]
    m8 = pl["consts"]["m8"]
    if eng == 'A':
        esc = pl["esc"].tile([P, 2, ROWS], FP8, tag="esc", name=f"esc{u}_{r}")
    for s, jc in enumerate(jcs):
        lg, q = jc // 4, jc % 4
        cc = _cc(kv, lg, q)
        pt = pl["pt"].tile([P, ROWS], F32, tag="pt", name=f"pt{u}{s}_{r}")
        lhsT = keys8[kv][:, :, jc * P:(jc + 1) * P]
        nc.tensor.matmul(pt[:, 0:CHUNK], lhsT, rows8[:, :, 0:CHUNK],
                         perf_mode=mybir.MatmulPerfMode.DoubleRow)
        nc.tensor.matmul(pt[:, CHUNK:ROWS], lhsT, rows8[:, :, CHUNK:ROWS],
                         perf_mode=mybir.MatmulPerfMode.DoubleRow)
        if lg == 0:
            # label logit: cross-view diag, rows of view (1-kv)
            dt = kv * 4 + jc
            dsc = pl["dsc"].tile([P, P], BF16, tag="dsc", name=f"dsc{u}{s}_{r}")
            nc.vector.scalar_tensor_tensor(
                dsc[:], pt[:, (1 - kv) * CHUNK + jc * P:
                            (1 - kv) * CHUNK + (jc + 1) * P],
                1.0, ident[:],
                op0=mybir.AluOpType.mult, op1=mybir.AluOpType.mult,
                accum_out=diag01[:, dt:dt + 1])
        if eng == 'A':
            nc.scalar.activation(esc[:, s, :], pt[:], AF.Exp,
                                 scale=sclT[:, cc:cc + 1], bias=bias5[:])
            if lg == 0:
                ds = kv * CHUNK + jc * P
                nc.gpsimd.tensor_mul(esc[:, s, ds:ds + P],
                                     esc[:, s, ds:ds + P], m8[:])
        else:
            it = pl["i16"].tile([P, ROWS], I16, tag="i16", name=f"it{u}{s}_{r}")
            nc.vector.tensor_scalar(
                it[:], pt[:], sAT[:, cc:cc + 1], SCH_B,
                op0=mybir.AluOpType.mult, op1=mybir.AluOpType.add)
            ebf = it[:].bitcast(BF16)
            if lg == 0:
                ds = kv * CHUNK + jc * P
                nc.gpsimd.tensor_mul(it[:, ds:ds + P].bitcast(BF16),
                                     it[:, ds:ds + P].bitcast(BF16), m8[:])
            nc.tensor.matmul(rsA[:], ones1b[:], ebf[:, 0:CHUNK],
                             start=first, stop=last)
            nc.tensor.matmul(rsB[:], ones1b[:], ebf[:, CHUNK:ROWS],
                             start=first, stop=last)
    if eng == 'A':
        nc.tensor.matmul(rsA[:], ones8_1[:], esc[:, :, 0:CHUNK],
                         perf_mode=mybir.MatmulPerfMode.DoubleRow,
                         start=first, stop=last)
        nc.tensor.matmul(rsB[:], ones8_1[:], esc[:, :, CHUNK:ROWS],
                         perf_mode=mybir.MatmulPerfMode.DoubleRow,
                         start=first, stop=last)


def _emit_epilogue(nc, pl, out_dram, r, rsA, rsB, diag01, sclT):
    ones_col = pl["consts"]["ones_col"]
    lnrA = pl["sml"].tile([1, CHUNK], F32, tag="lnrA", name=f"lnrA{r}")
    lnrB = pl["sml"].tile([1, CHUNK], F32, tag="lnrB", name=f"lnrB{r}")
    lnsA = pl["sml"].tile([1, 1], F32, tag="lnsA", name=f"lnsA{r}")
    lnsB = pl["sml"].tile([1, 1], F32, tag="lnsB", name=f"lnsB{r}")
    nc.scalar.activation(lnrA[:], rsA[0:1, :], AF.Ln, accum_out=lnsA[:])
    nc.scalar.activation(lnrB[:], rsB[0:1, :], AF.Ln, accum_out=lnsB[:])
    lab = pl["sml"].tile([P, 8], F32, tag="lab", name=f"lab{r}")
    nc.vector.tensor_mul(lab[:], diag01[:], sclT[:, 0:8])
    dsum = pl["sml"].tile([P, 1], F32, tag="dsum", name=f"dsum{r}")
    nc.vector.tensor_reduce(dsum[:], lab[:], axis=mybir.AxisListType.X,
                            op=mybir.AluOpType.add)
    fp = pl["pt"].tile([1, GL], F32, tag="pt", name=f"fp{r}")
    nc.tensor.matmul(fp[0:1, 0:1], dsum[:], ones_col[:])
    res = pl["sml"].tile([1, 2], F32, tag="res", name=f"res{r}")
    nc.vector.tensor_sub(res[:, 0:1], lnsA[:], fp[0:1, 0:1])
    nc.vector.tensor_add(res[:, 1:2], res[:, 0:1], lnsB[:])
    nc.sync.dma_start(out=out_dram[:], in_=res[:, 1:2])


def _emit_pass(nc, pl, vin, out_dram, r, do_setup=True, do_main=True,
               state_prev=None):
    """One full loss computation (rep r)."""
    if do_setup:
        raw = _emit_loads(nc, pl, vin, r)
        keys8 = [pl["nrm"].tile([P, KH, BN], FP8, tag=f"k8{v}",
                                name=f"k8{v}_{r}") for v in range(2)]
        ssc_sb = {}
        sclT = pl["sml"].tile([P, 64], F32, tag="sclT", name=f"sclT{r}")
        sAT = pl["sml"].tile([P, 64], F32, tag="sAT", name=f"sAT{r}")
        rows8 = pl["nrm"].tile([P, KH, ROWS], FP8, tag="rows8",
                               name=f"rows8_{r}")
        # group 0 first: normalize, rows8 (gates all logits), scale sg0
        ssb_bc = [pl["pt"].tile([P, GL], F32, tag="pt", name=f"ssbc{v}_{r}")
                  for v in range(2)]
        for v in range(2):
            _emit_norm_lg(nc, pl, r, v, 0, raw, keys8, ssc_sb, bc_out=ssb_bc)
        _emit_rows8(nc, pl, r, raw, ssb_bc, rows8)
        _emit_scale_sg(nc, pl, r, 0, ssc_sb, sclT, sAT)
        state = (keys8, rows8, sclT, sAT, ssc_sb, raw)
    else:
        state = state_prev
        keys8, rows8, sclT, sAT, ssc_sb, raw = state

    def _advance_setup(done_lg, want_lg):
        while done_lg < want_lg:
            done_lg += 1
            for v in range(2):
                _emit_norm_lg(nc, pl, r, v, done_lg, raw, keys8, ssc_sb)
            sg, _ = _LG_SG[done_lg]
            if done_lg == SG_LGS[sg][-1]:
                _emit_scale_sg(nc, pl, r, sg, ssc_sb, sclT, sAT)
        return done_lg

    if do_main:
        rsA = pl["rs"].tile([32, CHUNK], F32, tag="rsA", name=f"rsA{r}")
        rsB = pl["rs"].tile([32, CHUNK], F32, tag="rsB", name=f"rsB{r}")
        diag01 = pl["sml"].tile([P, 8], F32, tag="diag01", name=f"diag01{r}")
        done_lg = 0
        for u, (eng, kv, jcs) in enumerate(SCHEDULE):
            if do_setup:
                # emit normalize one load-group ahead of consumption so the
                # shared PSUM ring stays in time order
                want = min(max(jc // 4 for jc in jcs) + 1, LG - 1)
                done_lg = _advance_setup(done_lg, want)
            _emit_unit(nc, pl, r, u, eng, kv, jcs, keys8, rows8, sclT, sAT,
                       rsA, rsB, diag01, u == 0, u == len(SCHEDULE) - 1)
        _emit_epilogue(nc, pl, out_dram, r, rsA, rsB, diag01, sclT)
    elif do_setup:
        _advance_setup(0, LG - 1)
    return state


def _build_nc(reps: int = 1, mode: str = "full"):
    """mode: 'full' reps everything; 'main' reps only logits+exp (one
    shared setup); 'setup' reps only load+normalize."""
    nc = bacc.Bacc()
    vin = [
        nc.dram_tensor("view0", [B, E, N], F32, kind="ExternalInput"),
        nc.dram_tensor("view1", [B, E, N], F32, kind="ExternalInput"),
    ]
    ident_in = nc.dram_tensor("ident", [P, P], F32, kind="ExternalInput")
    out_dram = nc.dram_tensor("out", [1, 1], F32, kind="ExternalOutput")

    with ExitStack() as ctx:
        tc = ctx.enter_context(tile.TileContext(nc))
        pl = {
            name: ctx.enter_context(tc.tile_pool(name=name, bufs=bufs))
            for name, bufs in (("raw", 1), ("sq", 2), ("nrm", 1),
                               ("esc", 2), ("i16", 2), ("dsc", 2),
                               ("sml", 1))
        }
        pl["pt"] = ctx.enter_context(
            tc.tile_pool(name="pt", bufs=3, space="PSUM"))
        pl["rs"] = ctx.enter_context(
            tc.tile_pool(name="rs", bufs=1, space="PSUM"))

        # preload the one activation table that serves both Exp and Ln so
        # the implicit-table-load pass never inserts a reload
        tables = list(get_activation_tables(nc.m.arch).items())
        tidx = next(i for i, (nm, _) in enumerate(tables)
                    if nm == "natural_log_exp_and_others")
        nc.scalar.add_instruction(mybir.InstLoadActFuncSet(
            name=nc.get_next_instruction_name(), ins=[], outs=[],
            act_func_set_id=tidx))

        ident = pl["sml"].tile([P, P], F32, tag="ident", name="ident")
        nc.sync.dma_start(out=ident[:], in_=ident_in[:])
        consts = {"ident": ident}
        consts["ones8_1"] = pl["sml"].tile([P, KH, 32], FP8, tag="ones8_1",
                                           name="ones8_1")
        nc.vector.memset(consts["ones8_1"][:], 1.0)
        consts["ones8F"] = pl["sml"].tile([P, KH, P], FP8, tag="ones8F",
                                          name="ones8F")
        nc.vector.memset(consts["ones8F"][:], 1.0)
        consts["ones1b"] = pl["sml"].tile([P, 32], BF16, tag="ones1b",
                                          name="ones1b")
        nc.vector.memset(consts["ones1b"][:], 1.0)
        consts["onesP"] = pl["sml"].tile([1, P], BF16, tag="onesP",
                                         name="onesP")
        nc.vector.memset(consts["onesP"][:], 1.0)
        consts["ones_col"] = pl["sml"].tile([P, 1], F32, tag="ones_col",
                                            name="ones_col")
        nc.vector.memset(consts["ones_col"][:], 1.0)
        consts["bias5"] = pl["sml"].tile([P, 1], F32, tag="bias5",
                                         name="bias5")
        nc.vector.memset(consts["bias5"][:], EXPB)
        m8 = pl["sml"].tile([P, P], BF16, tag="m8", name="m8")
        nc.vector.memset(m8[:], 1.0)
        nc.vector.tensor_sub(m8[:], m8[:], ident[:])
        consts["m8"] = m8
        pl["consts"] = consts

        state = None
        for r in range(reps):
            state = _emit_pass(
                nc, pl, vin, out_dram, r,
                do_setup=(mode != "main" or r == 0),
                do_main=(mode != "setup"),
                state_prev=state)

    nc.compile()
    return nc


_NC_CACHE = None


def _run_spmd(view0: np.ndarray, view1: np.ndarray, nc=None, **spmd_kwargs):
    global _NC_CACHE
    if nc is None:
        if _NC_CACHE is None:
            _NC_CACHE = _build_nc()
        nc = _NC_CACHE

    ident = np.eye(P, dtype=np.float32)
    in_maps = []
    for c in range(NCORES):
        in_maps.append({
            "view0": np.ascontiguousarray(np.roll(view0, -c * (B // NCORES), axis=0)),
            "view1": np.ascontiguousarray(np.roll(view1, -c * (B // NCORES), axis=0)),
            "ident": ident,
        })
    res = run_bass_kernel_spmd(nc, in_maps, core_ids=list(range(NCORES)),
                               **spmd_kwargs)
    total = sum(float(r["out"][0, 0]) for r in res.results)
    # every nll term carries a +5 from the e^(l-5) tiles
    return np.float32(total / (2 * BN) - EXPB)


def kernel(view0: np.ndarray, view1: np.ndarray) -> np.ndarray:
    return _run_spmd(view0, view1)
